# revision 1
# baseline (speedup 1.0000x reference)
"""ContextualAttention TRN2 kernel.

Problem (B=4, C=64, H=W=64, K=Q=HW=4096):
    norm_bg = l2norm(bg, axis=C);  norm_fg = l2norm(fg, axis=C)
    att     = softmax_K(norm_bg^T @ norm_fg)        # [B, K, Q]
    out     = fg*(1-mask) + (bg @ att)*mask

Sharding: 8 cores = (batch b in 0..3) x (query half h in 0..1).
Each core sees the full key axis (K=4096) and Q=2048 queries, so the
softmax over K is core-local (no collectives).

Per-core algorithm (everything [C|K on partitions, HW on free]):
  - norms via ones-vector matmuls (partition reduction on PE),
    1/sqrt via exp(-0.5*ln(x)) on ScalarE (Rsqrt activation is banned),
    partition-broadcast of row vectors via ones-row matmuls.
  - scores s[k,q] = bgn^T @ fgn in float32r (1 cyc/row, ~1e-4 rel err),
    32 k-tiles x [128,512] per 512-wide q-tile, grouped 3 PSUM banks at
    a time so one Exp activation covers [128,1536] (ScalarE is the
    bottleneck engine: 8.4M exps/core).
  - softmax denominator for free: bgT is transposed WITH a ones row
    appended, so the re-weighting matmul acc[65,512] = bgT_aug^T @ exp_s
    accumulates both numerator (rows 0..63) and denominator (row 64).
  - epilogue per q-tile: recip on DVE, mask-fold, ones-row broadcast
    matmul, two tensor-tensor ops, DMA out.

This walrus build accepts at most ONE semaphore wait per instruction;
split_multiwaits() post-processes the BIR to hoist extra waits into
single-wait NoOps (see _fix_bir).
"""

import numpy as np

try:
    import concourse.bass as _bass  # noqa: F401
except ImportError:  # pragma: no cover - fallback for odd sys.path setups
    import sys
    for p in ("/opt/trn_rl_repo", "/root/.axon_site/_ro/trn_rl_repo"):
        if p not in sys.path:
            sys.path.insert(0, p)

B, C, H, W = 4, 64, 64, 64
K = H * W              # 4096 keys per batch
QH = K // 2            # 2048 queries per core
NCORES = 8
KT = K // 128          # 32 key tiles
QT = QH // 512         # 4 query tiles per core
GROUPS = [list(range(g * 3, min(KT, g * 3 + 3))) for g in range((KT + 2) // 3)]

_CACHE = {}


def _fix_bir(nc):
    """Hoist extra semaphore waits into single-wait NoOps (this walrus
    supports one wait per instruction) and pin the serialized BIR."""
    import orjson
    bir = orjson.loads(nc.to_json_bytes())
    ctr = 0
    for fn in bir["functions"]:
        for blk in fn["blocks"]:
            out = []
            for inst in blk.get("instructions", []):
                si = inst.get("sync_info")
                ow = (si or {}).get("on_wait") or []
                if len(ow) > 1:
                    for w in ow[:-1]:
                        ctr += 1
                        out.append({
                            "debug": inst.get("debug", 0),
                            "engine": inst["engine"], "ins": [],
                            "name": f"I-wsplit-{ctr}", "opcode": "NoOp",
                            "outs": [],
                            "sync_info": {"on_update": [], "on_wait": [w]},
                        })
                    si["on_wait"] = [ow[-1]]
                out.append(inst)
            blk["instructions"] = out
    fixed = orjson.dumps(bir)
    nc.to_json_bytes = lambda: fixed


def _build_nc():
    import concourse.bass as bass
    import concourse.mybir as mybir
    from concourse import tile

    f32 = mybir.dt.float32
    f32r = mybir.dt.float32r
    bf16 = mybir.dt.bfloat16
    AF = mybir.ActivationFunctionType
    OP = mybir.AluOpType
    mmdt = bf16

    nc = bass.Bass("TRN2", target_bir_lowering=False, debug=False)
    bg_d = nc.dram_tensor("bg", [C, K], f32, kind="ExternalInput")
    fg_d = nc.dram_tensor("fg", [C, QH], f32, kind="ExternalInput")
    mk_d = nc.dram_tensor("mk", [1, QH], f32, kind="ExternalInput")
    id_d = nc.dram_tensor("ident", [128, 128], f32, kind="ExternalInput")
    out_d = nc.dram_tensor("out", [C, QH], f32, kind="ExternalOutput")

    NG = KT // 2  # 16 groups of 2 k-tiles per q-tile

    with tile.TileContext(nc) as tc:
        with (
            tc.tile_pool(name="const", bufs=1) as constp,
            tc.tile_pool(name="sb", bufs=1) as sb,
            tc.tile_pool(name="expp", bufs=4) as expp,
            tc.tile_pool(name="outp", bufs=2) as outp,
            # single PSUM budget for the whole kernel (8 banks):
            #   score [128,1024] x2 = 4, acc [65,512] x2 = 2,
            #   n2 [1,512] = 1, rep/repq [64,512] = 1
            tc.tile_pool(name="mps", bufs=2, space="PSUM") as mps,
            tc.tile_pool(name="accp", bufs=2, space="PSUM") as accp,
            tc.tile_pool(name="n2p", bufs=1, space="PSUM") as n2p,
            tc.tile_pool(name="repp", bufs=1, space="PSUM") as repp,
        ):
            # ---- constants; dummy Ln/Exp prefetch the ACT table set ----
            dumf = constp.tile([1, 8], f32)
            nc.vector.memset(dumf[:], 1.0)
            dumo = constp.tile([1, 8], f32)
            nc.scalar.activation(dumo[:], dumf[:], AF.Ln)
            nc.scalar.activation(dumo[:], dumf[:], AF.Exp)
            ones_col_f = constp.tile([64, 1], f32)
            nc.vector.memset(ones_col_f[:], 1.0)
            ones_col = constp.tile([64, 1], f32r)
            nc.vector.tensor_copy(ones_col[:], ones_col_f[:])
            ones_row_f = constp.tile([1, 64], f32)
            nc.vector.memset(ones_row_f[:], 1.0)
            ones_row = constp.tile([1, 64], f32r)
            nc.vector.tensor_copy(ones_row[:], ones_row_f[:])
            idt = constp.tile([128, 128], f32)

            # ---- input DMAs: fg first (gates q-tile 0) ----
            fgs = sb.tile([64, QH], f32)
            for ch in range(2):
                nc.sync.dma_start(fgs[:, ch * 1024:(ch + 1) * 1024],
                                  fg_d[:, ch * 1024:(ch + 1) * 1024])
            nc.sync.dma_start(idt[:], id_d[:])
            mrow = sb.tile([1, QH], f32)
            nc.sync.dma_start(mrow[:], mk_d[:])
            bgxc = []
            for ch in range(4):
                t = sb.tile([65, 1024], f32, tag=f"bgx{ch}")
                nc.sync.dma_start(t[0:64, :], bg_d[:, ch * 1024:(ch + 1) * 1024])
                nc.vector.memset(t[64:65, :], 1.0)
                bgxc.append(t)

            sqf = sb.tile([64, QH], f32r)
            invn = sb.tile([1, K + QH], f32r)
            bgn = sb.tile([64, K], mmdt)
            fgn = sb.tile([64, QH], mmdt)
            fgm = sb.tile([64, QH], f32)
            bgT = sb.tile([128, KT * 65], mmdt)

            def norm_round(src_ap, dst_off):
                # 512-wide: ones-col matmul, then 1/sqrt = exp(-0.5*ln)
                n2 = n2p.tile([1, 512], f32, tag="n2")
                nc.tensor.matmul(n2[:], ones_col[:], src_ap,
                                 start=True, stop=True)
                lns = outp.tile([1, 512], f32, tag="lns")
                nc.scalar.activation(lns[:], n2[:], AF.Ln)
                nc.scalar.activation(invn[:, dst_off:dst_off + 512],
                                     lns[:], AF.Exp, scale=-0.5)

            def replicate_mul(dst, dst_off, src, src_off, inv_off):
                rep = repp.tile([64, 512], f32, tag="rep")
                nc.tensor.matmul(rep[:], ones_row[:],
                                 invn[0:1, inv_off:inv_off + 512],
                                 start=True, stop=True)
                nc.vector.tensor_mul(dst[:, dst_off:dst_off + 512],
                                     src[0:64, src_off:src_off + 512],
                                     rep[:])

            def bg_chunk_setup(ch):
                bx = bgxc[ch]
                sqb = sb.tile([64, 1024], f32r, tag=f"sqb{ch % 2}")
                nc.vector.tensor_mul(sqb[:], bx[0:64, :], bx[0:64, :])
                for j in range(2):
                    norm_round(sqb[:, j * 512:(j + 1) * 512],
                               ch * 1024 + j * 512)
                for j in range(8):
                    kt = ch * 8 + j
                    trps = mps.tile([128, 65], f32, tag="score")
                    nc.tensor.transpose(trps[:],
                                        bx[:, j * 128:(j + 1) * 128],
                                        idt[0:65, 0:65])
                    nc.vector.tensor_copy(bgT[:, kt * 65:(kt + 1) * 65],
                                          trps[:])
                for j in range(2):
                    replicate_mul(bgn, ch * 1024 + j * 512,
                                  bx, j * 512, ch * 1024 + j * 512)

            def group(qt, g, acc):
                q0 = qt * 512
                kts = [2 * g, 2 * g + 1]
                scp = mps.tile([128, 1024], f32, tag="score")
                for j, kt in enumerate(kts):
                    for h in range(2):
                        nc.tensor.matmul(
                            scp[:, j * 512 + h * 256:j * 512 + (h + 1) * 256],
                            bgn[:, kt * 128:(kt + 1) * 128],
                            fgn[:, q0 + h * 256:q0 + (h + 1) * 256],
                            start=True, stop=True)
                exg = expp.tile([128, 1024], mmdt, tag="exp")
                nc.scalar.activation(exg[:], scp[:], AF.Exp)
                for j, kt in enumerate(kts):
                    nc.tensor.matmul(
                        acc[:], bgT[:, kt * 65:kt * 65 + 65],
                        exg[:, j * 512:(j + 1) * 512],
                        start=(kt == 0), stop=(kt == KT - 1))

            def epilogue(qt, acc):
                q0 = qt * 512
                lnd = outp.tile([1, 512], f32, tag="lnd")
                nc.scalar.activation(lnd[:], acc[64:65, :], AF.Ln)
                rcp = outp.tile([1, 512], f32, tag="rcp")
                nc.scalar.activation(rcp[:], lnd[:], AF.Exp, scale=-1.0)
                mr = outp.tile([1, 512], f32r, tag="mr")
                nc.vector.tensor_mul(mr[:], rcp[:], mrow[0:1, q0:q0 + 512])
                repq = repp.tile([64, 512], f32, tag="rep")
                nc.tensor.matmul(repq[:], ones_row[:], mr[:],
                                 start=True, stop=True)
                rep_sb = outp.tile([64, 512], f32, tag="repsb")
                nc.vector.tensor_copy(rep_sb[:], repq[:])
                ot = outp.tile([64, 512], f32, tag="ot")
                nc.vector.tensor_mul(ot[:], acc[0:64, :], rep_sb[:])
                osb = outp.tile([64, 512], f32, tag="osb")
                nc.vector.tensor_add(osb[:], ot[:], fgm[:, q0:q0 + 512])
                nc.sync.dma_start(out_d[:, q0:q0 + 512], osb[:])

            # ---- fg pipeline (gates everything) ----
            for ch in range(2):
                sl = slice(ch * 1024, (ch + 1) * 1024)
                nc.vector.tensor_mul(sqf[:, sl], fgs[:, sl], fgs[:, sl])
                for j in range(2):
                    norm_round(sqf[:, ch * 1024 + j * 512:
                                    ch * 1024 + (j + 1) * 512],
                               K + ch * 1024 + j * 512)
            replicate_mul(fgn, 0, fgs, 0, K)

            # ---- q-tile 0 interleaved with bg chunk setup ----
            acc0 = accp.tile([65, 512], f32, tag="acc")
            for ch in range(4):
                bg_chunk_setup(ch)
                for g in range(4 * ch, 4 * ch + 4):
                    group(0, g, acc0)
            # remaining fg columns + mask terms (needed from epilogue 0 on)
            for ch in range(1, 4):
                replicate_mul(fgn, ch * 512, fgs, ch * 512, K + ch * 512)
            onem = sb.tile([1, QH], f32)
            nc.vector.tensor_scalar(onem[:], mrow[:], -1.0, 1.0,
                                    OP.mult, OP.add)
            onem_r = sb.tile([1, QH], f32r)
            nc.vector.tensor_copy(onem_r[:], onem[:])
            for ch in range(QT):
                rep = repp.tile([64, 512], f32, tag="rep")
                nc.tensor.matmul(rep[:], ones_row[:],
                                 onem_r[0:1, ch * 512:(ch + 1) * 512],
                                 start=True, stop=True)
                nc.vector.tensor_mul(fgm[:, ch * 512:(ch + 1) * 512],
                                     fgs[:, ch * 512:(ch + 1) * 512], rep[:])
            epilogue(0, acc0)

            # ---- q-tiles 1..3 ----
            for qt in range(1, QT):
                acc = accp.tile([65, 512], f32, tag="acc")
                for g in range(NG):
                    group(qt, g, acc)
                epilogue(qt, acc)

    _fix_bir(nc)
    return nc


def _shard_inputs(background, foreground, mask):
    ident = np.eye(128, dtype=np.float32)
    in_maps = []
    for i in range(NCORES):
        b, h = i // 2, i % 2
        qs = slice(h * QH, (h + 1) * QH)
        in_maps.append({
            "bg": np.ascontiguousarray(
                background[b].reshape(C, K).astype(np.float32)),
            "fg": np.ascontiguousarray(
                foreground[b].reshape(C, K)[:, qs].astype(np.float32)),
            "mk": np.ascontiguousarray(
                mask[b].reshape(1, K)[:, qs].astype(np.float32)),
            "ident": ident,
        })
    return in_maps


def _run(background, foreground, mask, **spmd_kwargs):
    from concourse.bass_utils import run_bass_kernel_spmd
    if "nc" not in _CACHE:
        _CACHE["nc"] = _build_nc()
    nc = _CACHE["nc"]
    in_maps = _shard_inputs(background, foreground, mask)
    res = run_bass_kernel_spmd(nc, in_maps, list(range(NCORES)),
                               **spmd_kwargs)
    out = np.empty((B, C, K), dtype=np.float32)
    for i in range(NCORES):
        b, h = i // 2, i % 2
        out[b, :, h * QH:(h + 1) * QH] = res.results[i]["out"]
    return out.reshape(B, C, H, W), res


def kernel(background, foreground, mask):
    out, _ = _run(background, foreground, mask)
    return out



# revision 6
# speedup vs baseline: 1.3255x; 1.3255x over previous
"""ContextualAttention TRN2 kernel — mask-sparse + fp8 DoubleRow rewrite.

Problem (B=4, C=64, H=W=64, K=HW=4096):
    norm_bg = l2norm(bg, axis=C);  norm_fg = l2norm(fg, axis=C)
    att     = softmax_K(norm_bg^T @ norm_fg)        # [B, K, Q]
    out     = fg*(1-mask) + (bg @ att)*mask

Key structural ideas vs the dense baseline:
  * Mask sparsity: the output uses attended values ONLY where mask==1
    (~2036 of 4096 queries per batch).  The host gathers the masked
    query columns, the device computes attention just for those, and
    the host scatters results into a copy of `foreground` (for mask==0
    the output IS foreground).  This halves every device cost.
  * Sharding: core = (batch b, half h); each core sees the full key
    axis (softmax core-local) and up to QCAP=1152 gathered queries.
  * fp8e4 DoubleRow matmuls (2 contraction tiles per streamed column,
    0.5 cyc/col) for both big matmul groups:
      - scores:   lhsT bg8n[32,(2,128)] (C split 2x32), rhs
                  fgn8[32,(2,W)], out [128,W]
      - attended: lhsT bgT8p[128,(2,128)] (kt pairs; 65 used cols
                  zero-padded to 128 — walrus requires stationary
                  width 128 for DoubleRow), rhs exg[128,(2,W)]
    The exp output layout [kt0|kt1|kt2|kt3] per scp tile is exactly
    the DoubleRow ifmap layout.
  * Softmax denominator from the ones-column folded into bgT (row 64).
  * bg inverse norms computed partition-parallel: n2 via
    scalar_tensor_tensor square-accumulate on the transposed tiles
    ([128,32] layout), inv-sqrt = exp(-0.5*ln) on ACT at [128,8] tiles,
    flattened to a row via transpose + SBUF->SBUF DMA.
  * Division by the softmax denominator via DVE reciprocal (accurate)
    instead of Ln/Exp on the scalar engine.

Walrus quirks honored: one semaphore wait per instruction
(split_multiwaits post-pass), DVE ops read at most one PSUM operand,
DoubleRow stationary free width must be 128.
"""

import numpy as np

try:
    import concourse.bass as _bass  # noqa: F401
except ImportError:  # pragma: no cover - fallback for odd sys.path setups
    import sys
    for p in ("/opt/trn_rl_repo", "/root/.axon_site/_ro/trn_rl_repo"):
        if p not in sys.path:
            sys.path.insert(0, p)

B, C, H, W = 4, 64, 64, 64
K = H * W               # 4096 keys per batch
KT = K // 128           # 32 key tiles
NCH = 4                 # bg chunks of 1024 keys (8 kt each)
QCAP = 1152             # per-core query capacity (count_b <= 2304)
QTILES = [(0, 256), (256, 256), (512, 256), (768, 256), (1024, 128)]
NCORES = 8

_CACHE = {}


def _fix_bir(nc):
    """Hoist extra semaphore waits into single-wait NoOps (this walrus
    supports one wait per instruction) and pin the serialized BIR."""
    import orjson
    bir = orjson.loads(nc.to_json_bytes())
    ctr = 0
    for fn in bir["functions"]:
        for blk in fn["blocks"]:
            out = []
            for inst in blk.get("instructions", []):
                si = inst.get("sync_info")
                ow = (si or {}).get("on_wait") or []
                if len(ow) > 1:
                    for w in ow[:-1]:
                        ctr += 1
                        out.append({
                            "debug": inst.get("debug", 0),
                            "engine": inst["engine"], "ins": [],
                            "name": f"I-wsplit-{ctr}", "opcode": "NoOp",
                            "outs": [],
                            "sync_info": {"on_update": [], "on_wait": [w]},
                        })
                    si["on_wait"] = [ow[-1]]
                out.append(inst)
            blk["instructions"] = out
    fixed = orjson.dumps(bir)
    nc.to_json_bytes = lambda: fixed


def _build_nc():
    import concourse.bass as bass
    import concourse.mybir as mybir
    from concourse import tile

    f32 = mybir.dt.float32
    f32r = mybir.dt.float32r
    fp8 = mybir.dt.float8e4
    AF = mybir.ActivationFunctionType
    OP = mybir.AluOpType
    PM = mybir.MatmulPerfMode

    nc = bass.Bass("TRN2", target_bir_lowering=False, debug=False)
    bg_d = nc.dram_tensor("bg", [C, K], f32, kind="ExternalInput")
    fg_d = nc.dram_tensor("fg", [C, QCAP], f32, kind="ExternalInput")
    id_d = nc.dram_tensor("ident", [128, 128], f32, kind="ExternalInput")
    out_d = nc.dram_tensor("out", [C, QCAP], f32, kind="ExternalOutput")

    with tile.TileContext(nc) as tc:
        with (
            tc.tile_pool(name="const", bufs=1) as constp,
            tc.tile_pool(name="sb", bufs=1) as sb,
            tc.tile_pool(name="expp", bufs=3) as expp,
            tc.tile_pool(name="outp", bufs=2) as outp,
            # PSUM budget (8 banks): scp 2x2 + acc 2x1 + aux 2x1
            tc.tile_pool(name="scps", bufs=2, space="PSUM") as scps,
            tc.tile_pool(name="accp", bufs=2, space="PSUM") as accp,
            tc.tile_pool(name="auxp", bufs=2, space="PSUM") as auxp,
        ):
            # ---- constants; dummy Ln/Exp prefetch the ACT table set ----
            dumf = constp.tile([1, 8], f32)
            nc.vector.memset(dumf[:], 1.0)
            dumo = constp.tile([1, 8], f32)
            nc.scalar.activation(dumo[:], dumf[:], AF.Ln)
            nc.scalar.activation(dumo[:], dumf[:], AF.Exp)
            ones_col_f = constp.tile([64, 1], f32)
            nc.vector.memset(ones_col_f[:], 1.0)
            ones_col = constp.tile([64, 1], f32r)
            nc.vector.tensor_copy(ones_col[:], ones_col_f[:])
            ones_row_f = constp.tile([1, 64], f32)
            nc.vector.memset(ones_row_f[:], 1.0)
            ones_row = constp.tile([1, 64], f32r)
            nc.vector.tensor_copy(ones_row[:], ones_row_f[:])
            idt = constp.tile([128, 128], f32)

            # ---- input DMAs: fg first (gates q-pipeline) ----
            fgs = sb.tile([64, QCAP], f32)
            nc.sync.dma_start(fgs[:, 0:576], fg_d[:, 0:576])
            nc.sync.dma_start(fgs[:, 576:QCAP], fg_d[:, 576:QCAP])
            nc.sync.dma_start(idt[:], id_d[:])

            # ---- persistent SBUF tensors ----
            fgn8 = sb.tile([32, 2 * QCAP], fp8)
            bg8n = sb.tile([32, 2 * K], fp8)
            bgT8p = sb.tile([128, KT * 128], fp8)
            n2b = sb.tile([128, KT], f32)
            invb = sb.tile([128, KT], f32)
            invrow = sb.tile([1, K], f32r)
            invf = sb.tile([1, QCAP], f32r)
            sq = sb.tile([128, 64], f32)

            # zero the 63-wide pads of every bgT8p block (cols 65..127)
            pads = bgT8p[:, :].rearrange("p (kt c) -> p kt c", kt=KT)
            nc.vector.memset(pads[:, :, 65:128], 0.0)

            # ---- fg pipeline: normalize + fp8 in split layout ----
            sqf = sb.tile([64, QCAP], f32r)
            nc.vector.tensor_mul(sqf[:], fgs[:], fgs[:])
            FWIN = [(0, 512), (512, 512), (1024, 128)]
            for q0, w in FWIN:
                n2f = auxp.tile([1, 512], f32, tag="aux")
                nc.tensor.matmul(n2f[0:1, 0:w], ones_col[:],
                                 sqf[:, q0:q0 + w], start=True, stop=True)
                lns = outp.tile([1, 512], f32, tag="lns")
                nc.scalar.activation(lns[0:1, 0:w], n2f[0:1, 0:w], AF.Ln)
                nc.scalar.activation(invf[:, q0:q0 + w], lns[0:1, 0:w],
                                     AF.Exp, scale=-0.5)
            for q0, w in FWIN:
                repf = auxp.tile([64, 512], f32, tag="aux")
                nc.tensor.matmul(repf[:, 0:w], ones_row[:],
                                 invf[0:1, q0:q0 + w], start=True, stop=True)
                nc.vector.tensor_mul(fgn8[:, q0:q0 + w],
                                     fgs[0:32, q0:q0 + w], repf[0:32, 0:w])
                nc.vector.tensor_mul(fgn8[:, QCAP + q0:QCAP + q0 + w],
                                     fgs[32:64, q0:q0 + w], repf[32:64, 0:w])

            fgn8a = fgn8[:, :].rearrange("p (two q) -> p two q", two=2)
            bg8na = bg8n[:, :].rearrange("p (two k) -> p two k", two=2)

            acc0 = accp.tile([128, 256], f32, tag="acc")

            def subgroup(qt, sg, acc):
                """Scores + exp + attended for kts 4sg..4sg+3, one qtile."""
                q0, w = QTILES[qt]
                scp = scps.tile([128, 4 * 256], f32, tag="scp")
                for j in range(4):
                    kt = 4 * sg + j
                    nc.tensor.matmul(
                        scp[:, j * w:(j + 1) * w],
                        bg8na[:, :, kt * 128:(kt + 1) * 128],
                        fgn8a[:, :, q0:q0 + w],
                        start=True, stop=True, perf_mode=PM.DoubleRow)
                exg = expp.tile([128, 4 * 256], fp8, tag="exp")
                nc.scalar.activation(exg[:, 0:4 * w], scp[:, 0:4 * w], AF.Exp)
                for g2 in range(2):
                    a = 4 * sg + 2 * g2
                    lhsT = bgT8p[:, a * 128:(a + 2) * 128].rearrange(
                        "p (two c) -> p two c", two=2)
                    rhs = exg[:, 2 * g2 * w:2 * g2 * w + 2 * w].rearrange(
                        "p (two q) -> p two q", two=2)
                    nc.tensor.matmul(acc[:, 0:w], lhsT, rhs,
                                     start=(sg == 0 and g2 == 0),
                                     stop=(sg == 7 and g2 == 1),
                                     perf_mode=PM.DoubleRow)

            # ---- bg chunks interleaved with qtile-0 subgroups ----
            for ch in range(NCH):
                bgx = sb.tile([65, 1024], f32, tag=f"bgx{ch}")
                nc.sync.dma_start(bgx[0:64, :],
                                  bg_d[:, ch * 1024:(ch + 1) * 1024])
                nc.vector.memset(bgx[64:65, :], 1.0)
                for j in range(8):
                    kt = 8 * ch + j
                    trp = auxp.tile([128, 65], f32, tag="aux")
                    nc.tensor.transpose(trp[:, 0:65],
                                        bgx[:, j * 128:(j + 1) * 128],
                                        idt[0:65, 0:65])
                    nc.vector.tensor_copy(
                        bgT8p[:, kt * 128:kt * 128 + 65], trp[:, 0:65])
                    trs = outp.tile([128, 64], f32, tag="trs")
                    nc.vector.tensor_copy(trs[:], trp[:, 0:64])
                    nc.vector.scalar_tensor_tensor(
                        out=sq[:], in0=trs[:], scalar=1.0, in1=trs[:],
                        op0=OP.mult, op1=OP.mult,
                        accum_out=n2b[:, kt:kt + 1])
                # inv-sqrt of this chunk's 8 key-tile norm columns
                lnb = outp.tile([128, 8], f32, tag="lnb")
                nc.scalar.activation(lnb[:], n2b[:, 8 * ch:8 * ch + 8], AF.Ln)
                nc.scalar.activation(invb[:, 8 * ch:8 * ch + 8], lnb[:],
                                     AF.Exp, scale=-0.5)
                # flatten to row layout: transpose + sbuf->sbuf dma
                ibt = auxp.tile([8, 128], f32, tag="aux")
                nc.tensor.transpose(ibt[:], invb[:, 8 * ch:8 * ch + 8],
                                    idt[:, 0:128])
                ibs = outp.tile([8, 128], f32r, tag="ibs")
                nc.vector.tensor_copy(ibs[:], ibt[:])
                nc.sync.dma_start(invrow[0:1, ch * 1024:(ch + 1) * 1024],
                                  ibs[:])
                # normalized fp8 bg in split-C layout
                for v in range(2):
                    k0 = ch * 1024 + v * 512
                    repb = auxp.tile([64, 512], f32, tag="aux")
                    nc.tensor.matmul(repb[:], ones_row[:],
                                     invrow[0:1, k0:k0 + 512],
                                     start=True, stop=True)
                    nc.vector.tensor_mul(bg8n[:, k0:k0 + 512],
                                         bgx[0:32, v * 512:(v + 1) * 512],
                                         repb[0:32, :])
                    nc.vector.tensor_mul(bg8n[:, K + k0:K + k0 + 512],
                                         bgx[32:64, v * 512:(v + 1) * 512],
                                         repb[32:64, :])
                # qtile-0 work for the two fresh subgroups
                subgroup(0, 2 * ch, acc0)
                subgroup(0, 2 * ch + 1, acc0)

            def epilogue(qt, acc):
                q0, w = QTILES[qt]
                rcp = outp.tile([1, 256], f32r, tag="rcp")
                with nc.allow_low_precision(reason="f32r is bit-identical f32"):
                    nc.vector.reciprocal(rcp[0:1, 0:w], acc[64:65, 0:w])
                repq = auxp.tile([64, 512], f32, tag="aux")
                nc.tensor.matmul(repq[:, 0:w], ones_row[:], rcp[0:1, 0:w],
                                 start=True, stop=True)
                reps = outp.tile([64, 256], f32, tag="reps")
                nc.vector.tensor_copy(reps[:, 0:w], repq[:, 0:w])
                osb = outp.tile([64, 256], f32, tag="osb")
                nc.vector.tensor_mul(osb[:, 0:w], acc[0:64, 0:w],
                                     reps[:, 0:w])
                nc.sync.dma_start(out_d[:, q0:q0 + w], osb[:, 0:w])

            epilogue(0, acc0)

            # ---- qtiles 1..4 ----
            for qt in range(1, len(QTILES)):
                acc = accp.tile([128, 256], f32, tag="acc")
                for sg in range(8):
                    subgroup(qt, sg, acc)
                epilogue(qt, acc)

    _fix_bir(nc)
    return nc


def _shard_inputs(background, foreground, mask):
    ident = np.eye(128, dtype=np.float32)
    bgf = background.reshape(B, C, K).astype(np.float32)
    fgf = foreground.reshape(B, C, K).astype(np.float32)
    mkf = mask.reshape(B, K)
    in_maps = []
    scatter = []
    for b in range(B):
        idx = np.nonzero(mkf[b] > 0.5)[0]
        n = len(idx)
        assert n <= 2 * QCAP, f"masked count {n} exceeds capacity"
        n0 = (n + 1) // 2
        for h, part in enumerate((idx[:n0], idx[n0:])):
            sel = np.zeros(QCAP, dtype=np.int64)
            sel[:len(part)] = part
            in_maps.append({
                "bg": np.ascontiguousarray(bgf[b]),
                "fg": np.ascontiguousarray(fgf[b][:, sel]),
                "ident": ident,
            })
            scatter.append((b, part))
    return in_maps, scatter


def _run(background, foreground, mask, **spmd_kwargs):
    from concourse.bass_utils import run_bass_kernel_spmd
    if "nc" not in _CACHE:
        _CACHE["nc"] = _build_nc()
    nc = _CACHE["nc"]
    in_maps, scatter = _shard_inputs(background, foreground, mask)
    res = run_bass_kernel_spmd(nc, in_maps, list(range(NCORES)),
                               **spmd_kwargs)
    out = foreground.reshape(B, C, K).astype(np.float32).copy()
    for i in range(NCORES):
        b, part = scatter[i]
        if len(part):
            out[b][:, part] = res.results[i]["out"][:, :len(part)]
    return out.reshape(B, C, H, W), res


def kernel(background, foreground, mask):
    out, _ = _run(background, foreground, mask)
    return out


# revision 7
# speedup vs baseline: 1.3644x; 1.0294x over previous
"""ContextualAttention TRN2 kernel — mask-sparse + fp8 DoubleRow.

Problem (B=4, C=64, H=W=64, K=HW=4096):
    norm_bg = l2norm(bg, axis=C);  norm_fg = l2norm(fg, axis=C)
    att     = softmax_K(norm_bg^T @ norm_fg)        # [B, K, Q]
    out     = fg*(1-mask) + (bg @ att)*mask

Structure:
  * Mask sparsity: attended values are only needed where mask==1
    (~2036/4096 queries per batch).  The host gathers those columns,
    the device runs attention for them alone, and the host scatters
    results into a copy of `foreground` — for mask==0 the output IS
    foreground.  This halves all device work.
  * Sharding: core = (batch, half); full key axis per core (softmax is
    core-local), up to QCAP=1152 gathered queries per core.
  * fp8e4 DoubleRow matmuls for both big GEMMs (2 contraction tiles
    per streamed column):
      scores:   lhsT bg8n[32,(2,128)] (C split 2x32) x fgn8[32,(2,W)]
      attended: lhsT bgT8p[128,(2,128)] (kt pairs, 65 used columns
                zero-padded to 128: walrus requires stationary width
                128 for DoubleRow) x exg[128,(2,W)]
    The exp tile layout [kt0|kt1|kt2|kt3] is already the DoubleRow
    ifmap layout.
  * Softmax denominator via ones-column folded into bgT (row 64).
  * bg inverse norms partition-parallel: DVE square-accumulate over
    the fp8 transposed tiles -> [128,32]; inv-sqrt=exp(-0.5*ln) on ACT;
    row layout via transpose + SBUF->SBUF DMA flatten.
  * Engine schedule: scores emitted one subgroup ahead of the attended
    matmuls so the PE never stalls on the exp; each epilogue is emitted
    after the next qtile's first score group (the slow DVE reciprocal
    runs off the PE critical path); GPSIMD takes memsets + fg squares.

Walrus quirks honored: one semaphore wait per instruction
(split_multiwaits post-pass), DVE ops read at most one PSUM operand,
DoubleRow stationary width must be 128, DVE partition offsets must be
multiples of 32.
"""

import numpy as np

try:
    import concourse.bass as _bass  # noqa: F401
except ImportError:  # pragma: no cover - fallback for odd sys.path setups
    import sys
    for p in ("/opt/trn_rl_repo", "/root/.axon_site/_ro/trn_rl_repo"):
        if p not in sys.path:
            sys.path.insert(0, p)

B, C, H, W = 4, 64, 64, 64
K = H * W               # 4096 keys per batch
KT = K // 128           # 32 key tiles
NCH = 4                 # bg chunks of 1024 keys (8 kt each)
QCAP = 1152             # per-core query capacity (count_b <= 2304)
QTILES = [(0, 256), (256, 256), (512, 256), (768, 256), (1024, 128)]
NCORES = 8

_CACHE = {}


def _fix_bir(nc):
    """Hoist extra semaphore waits into single-wait NoOps (this walrus
    supports one wait per instruction) and pin the serialized BIR."""
    import orjson
    bir = orjson.loads(nc.to_json_bytes())
    ctr = 0
    for fn in bir["functions"]:
        for blk in fn["blocks"]:
            out = []
            for inst in blk.get("instructions", []):
                si = inst.get("sync_info")
                ow = (si or {}).get("on_wait") or []
                if len(ow) > 1:
                    for w in ow[:-1]:
                        ctr += 1
                        out.append({
                            "debug": inst.get("debug", 0),
                            "engine": inst["engine"], "ins": [],
                            "name": f"I-wsplit-{ctr}", "opcode": "NoOp",
                            "outs": [],
                            "sync_info": {"on_update": [], "on_wait": [w]},
                        })
                    si["on_wait"] = [ow[-1]]
                out.append(inst)
            blk["instructions"] = out
    fixed = orjson.dumps(bir)
    nc.to_json_bytes = lambda: fixed


def _build_nc():
    import concourse.bass as bass
    import concourse.mybir as mybir
    from concourse import tile

    f32 = mybir.dt.float32
    f32r = mybir.dt.float32r
    fp8 = mybir.dt.float8e4
    AF = mybir.ActivationFunctionType
    OP = mybir.AluOpType
    PM = mybir.MatmulPerfMode

    nc = bass.Bass("TRN2", target_bir_lowering=False, debug=False)
    bg_d = nc.dram_tensor("bg", [C, K], f32, kind="ExternalInput")
    fg_d = nc.dram_tensor("fg", [C, QCAP], f32, kind="ExternalInput")
    id_d = nc.dram_tensor("ident", [128, 128], f32, kind="ExternalInput")
    out_d = nc.dram_tensor("out", [C, QCAP], f32, kind="ExternalOutput")

    with tile.TileContext(nc) as tc:
        with (
            tc.tile_pool(name="const", bufs=1) as constp,
            tc.tile_pool(name="sb", bufs=1) as sb,
            tc.tile_pool(name="expp", bufs=3) as expp,
            tc.tile_pool(name="outp", bufs=2) as outp,
            # PSUM budget (8 banks): scp 2x2 + acc 2x1 + aux 2x1
            tc.tile_pool(name="scps", bufs=2, space="PSUM") as scps,
            tc.tile_pool(name="accp", bufs=2, space="PSUM") as accp,
            tc.tile_pool(name="auxp", bufs=2, space="PSUM") as auxp,
        ):
            # ---- constants; dummy Ln/Exp prefetch the ACT table set ----
            dumf = constp.tile([1, 8], f32)
            nc.vector.memset(dumf[:], 1.0)
            dumo = constp.tile([1, 8], f32)
            nc.scalar.activation(dumo[:], dumf[:], AF.Ln)
            nc.scalar.activation(dumo[:], dumf[:], AF.Exp)
            ones_col_f = constp.tile([64, 1], f32)
            nc.vector.memset(ones_col_f[:], 1.0)
            ones_col = constp.tile([64, 1], f32r)
            nc.vector.tensor_copy(ones_col[:], ones_col_f[:])
            ones_row_f = constp.tile([1, 64], f32)
            nc.vector.memset(ones_row_f[:], 1.0)
            ones_row = constp.tile([1, 64], f32r)
            nc.vector.tensor_copy(ones_row[:], ones_row_f[:])
            idt = constp.tile([128, 128], f32)

            # ---- input DMAs: fg first (gates q-pipeline) ----
            fgs = sb.tile([64, QCAP], f32)
            nc.sync.dma_start(fgs[:, 0:576], fg_d[:, 0:576])
            nc.sync.dma_start(fgs[:, 576:QCAP], fg_d[:, 576:QCAP])
            nc.sync.dma_start(idt[:], id_d[:])

            # ---- persistent SBUF tensors ----
            fgn8 = sb.tile([32, 2 * QCAP], fp8)
            bg8n = sb.tile([32, 2 * K], fp8)
            bgT8p = sb.tile([128, KT * 128], fp8)
            n2b = sb.tile([128, KT], f32)
            invb = sb.tile([128, KT], f32)
            invrow = sb.tile([1, K], f32r)
            invf = sb.tile([1, QCAP], f32r)
            sq = sb.tile([128, 64], f32)

            # zero the 63-wide pads of every bgT8p block (cols 65..127)
            pads = bgT8p[:, :].rearrange("p (kt c) -> p kt c", kt=KT)
            nc.gpsimd.memset(pads[:, :, 65:128], 0.0)

            # bg chunk DMAs up front (async), ones rows on gpsimd
            bgxc = []
            for ch in range(NCH):
                bgx = sb.tile([65, 1024], f32, tag=f"bgx{ch}")
                nc.sync.dma_start(bgx[0:64, :],
                                  bg_d[:, ch * 1024:(ch + 1) * 1024])
                nc.gpsimd.memset(bgx[64:65, :], 1.0)
                bgxc.append(bgx)

            # ---- fg pipeline: normalize + fp8 in split layout ----
            sqf = sb.tile([64, QCAP], f32r)
            nc.gpsimd.tensor_mul(sqf[:], fgs[:], fgs[:])
            FWIN = [(0, 512), (512, 512), (1024, 128)]
            for q0, w in FWIN:
                n2f = auxp.tile([1, 512], f32, tag="aux")
                nc.tensor.matmul(n2f[0:1, 0:w], ones_col[:],
                                 sqf[:, q0:q0 + w], start=True, stop=True)
                lns = outp.tile([1, 512], f32, tag="lns")
                nc.scalar.activation(lns[0:1, 0:w], n2f[0:1, 0:w], AF.Ln)
                nc.scalar.activation(invf[:, q0:q0 + w], lns[0:1, 0:w],
                                     AF.Exp, scale=-0.5)
            for q0, w in FWIN:
                repf = auxp.tile([64, 512], f32, tag="aux")
                nc.tensor.matmul(repf[:, 0:w], ones_row[:],
                                 invf[0:1, q0:q0 + w], start=True, stop=True)
                nc.vector.tensor_mul(fgn8[:, q0:q0 + w],
                                     fgs[0:32, q0:q0 + w], repf[0:32, 0:w])
                nc.vector.tensor_mul(fgn8[:, QCAP + q0:QCAP + q0 + w],
                                     fgs[32:64, q0:q0 + w], repf[32:64, 0:w])

            fgn8a = fgn8[:, :].rearrange("p (two q) -> p two q", two=2)
            bg8na = bg8n[:, :].rearrange("p (two k) -> p two k", two=2)

            def sc_part(qt, sg):
                """Score DoubleRow matmuls for kts 4sg..4sg+3 at qtile qt."""
                q0, w = QTILES[qt]
                scp = scps.tile([128, 4 * 256], f32, tag="scp")
                for j in range(4):
                    kt = 4 * sg + j
                    nc.tensor.matmul(
                        scp[:, j * w:(j + 1) * w],
                        bg8na[:, :, kt * 128:(kt + 1) * 128],
                        fgn8a[:, :, q0:q0 + w],
                        start=True, stop=True, perf_mode=PM.DoubleRow)
                return scp

            def ea_part(qt, sg, scp, acc):
                """Exp + attended DoubleRow for subgroup sg."""
                q0, w = QTILES[qt]
                exg = expp.tile([128, 4 * 256], fp8, tag="exp")
                nc.scalar.activation(exg[:, 0:4 * w], scp[:, 0:4 * w], AF.Exp)
                for g2 in range(2):
                    a = 4 * sg + 2 * g2
                    lhsT = bgT8p[:, a * 128:(a + 2) * 128].rearrange(
                        "p (two c) -> p two c", two=2)
                    rhs = exg[:, 2 * g2 * w:2 * g2 * w + 2 * w].rearrange(
                        "p (two q) -> p two q", two=2)
                    nc.tensor.matmul(acc[:, 0:w], lhsT, rhs,
                                     start=(sg == 0 and g2 == 0),
                                     stop=(sg == 7 and g2 == 1),
                                     perf_mode=PM.DoubleRow)

            def chunk_setup(ch):
                bgx = bgxc[ch]
                for j in range(8):
                    kt = 8 * ch + j
                    trp = auxp.tile([128, 65], f32, tag="aux")
                    nc.tensor.transpose(trp[:, 0:65],
                                        bgx[:, j * 128:(j + 1) * 128],
                                        idt[0:65, 0:65])
                    nc.vector.tensor_copy(
                        bgT8p[:, kt * 128:kt * 128 + 65], trp[:, 0:65])
                    nc.vector.scalar_tensor_tensor(
                        out=sq[:], in0=bgT8p[:, kt * 128:kt * 128 + 64],
                        scalar=1.0, in1=bgT8p[:, kt * 128:kt * 128 + 64],
                        op0=OP.mult, op1=OP.mult,
                        accum_out=n2b[:, kt:kt + 1])
                # inv-sqrt of this chunk's 8 key-tile norm columns
                lnb = outp.tile([128, 8], f32, tag="lnb")
                nc.scalar.activation(lnb[:], n2b[:, 8 * ch:8 * ch + 8], AF.Ln)
                nc.scalar.activation(invb[:, 8 * ch:8 * ch + 8], lnb[:],
                                     AF.Exp, scale=-0.5)
                # flatten to row layout: transpose + sbuf->sbuf dma
                ibt = auxp.tile([8, 128], f32, tag="aux")
                nc.tensor.transpose(ibt[:], invb[:, 8 * ch:8 * ch + 8],
                                    idt[:, 0:128])
                ibs = outp.tile([8, 128], f32r, tag="ibs")
                nc.vector.tensor_copy(ibs[:], ibt[:])
                nc.sync.dma_start(invrow[0:1, ch * 1024:(ch + 1) * 1024],
                                  ibs[:])
                # normalized fp8 bg in split-C layout
                for v in range(2):
                    k0 = ch * 1024 + v * 512
                    repb = auxp.tile([64, 512], f32, tag="aux")
                    nc.tensor.matmul(repb[:], ones_row[:],
                                     invrow[0:1, k0:k0 + 512],
                                     start=True, stop=True)
                    nc.vector.tensor_mul(bg8n[:, k0:k0 + 512],
                                         bgx[0:32, v * 512:(v + 1) * 512],
                                         repb[0:32, :])
                    nc.vector.tensor_mul(bg8n[:, K + k0:K + k0 + 512],
                                         bgx[32:64, v * 512:(v + 1) * 512],
                                         repb[32:64, :])

            def epilogue(qt, acc):
                q0, w = QTILES[qt]
                rcp = outp.tile([1, 256], f32r, tag="rcp")
                with nc.allow_low_precision(reason="f32r is bit-same f32"):
                    nc.vector.reciprocal(rcp[0:1, 0:w], acc[64:65, 0:w])
                repq = auxp.tile([64, 512], f32, tag="aux")
                nc.tensor.matmul(repq[:, 0:w], ones_row[:], rcp[0:1, 0:w],
                                 start=True, stop=True)
                reps = outp.tile([64, 256], f32, tag="reps")
                nc.vector.tensor_copy(reps[:, 0:w], repq[:, 0:w])
                osb = outp.tile([64, 256], f32, tag="osb")
                nc.vector.tensor_mul(osb[:, 0:w], acc[0:64, 0:w],
                                     reps[:, 0:w])
                nc.sync.dma_start(out_d[:, q0:q0 + w], osb[:, 0:w])

            # ---- qtile 0 interleaved with bg chunk setup ----
            acc0 = accp.tile([128, 256], f32, tag="acc")
            for ch in range(NCH):
                chunk_setup(ch)
                sc_a = sc_part(0, 2 * ch)
                sc_b = sc_part(0, 2 * ch + 1)
                ea_part(0, 2 * ch, sc_a, acc0)
                ea_part(0, 2 * ch + 1, sc_b, acc0)

            # ---- qtiles 1..4, scores one subgroup ahead; epilogues are
            # emitted after the NEXT qtile's first score group ----
            pending = (0, acc0)
            for qt in range(1, len(QTILES)):
                acc = accp.tile([128, 256], f32, tag="acc")
                scp_cur = sc_part(qt, 0)
                if pending is not None:
                    epilogue(*pending)
                for sg in range(8):
                    scp_next = sc_part(qt, sg + 1) if sg < 7 else None
                    ea_part(qt, sg, scp_cur, acc)
                    scp_cur = scp_next
                pending = (qt, acc)
            epilogue(*pending)

    _fix_bir(nc)
    return nc


def _shard_inputs(background, foreground, mask):
    ident = np.eye(128, dtype=np.float32)
    bgf = background.reshape(B, C, K).astype(np.float32)
    fgf = foreground.reshape(B, C, K).astype(np.float32)
    mkf = mask.reshape(B, K)
    in_maps = []
    scatter = []
    for b in range(B):
        idx = np.nonzero(mkf[b] > 0.5)[0]
        n = len(idx)
        assert n <= 2 * QCAP, f"masked count {n} exceeds capacity"
        n0 = (n + 1) // 2
        for h, part in enumerate((idx[:n0], idx[n0:])):
            sel = np.zeros(QCAP, dtype=np.int64)
            sel[:len(part)] = part
            in_maps.append({
                "bg": np.ascontiguousarray(bgf[b]),
                "fg": np.ascontiguousarray(fgf[b][:, sel]),
                "ident": ident,
            })
            scatter.append((b, part))
    return in_maps, scatter


def _run(background, foreground, mask, **spmd_kwargs):
    from concourse.bass_utils import run_bass_kernel_spmd
    if "nc" not in _CACHE:
        _CACHE["nc"] = _build_nc()
    nc = _CACHE["nc"]
    in_maps, scatter = _shard_inputs(background, foreground, mask)
    res = run_bass_kernel_spmd(nc, in_maps, list(range(NCORES)),
                               **spmd_kwargs)
    out = foreground.reshape(B, C, K).astype(np.float32).copy()
    for i in range(NCORES):
        b, part = scatter[i]
        if len(part):
            out[b][:, part] = res.results[i]["out"][:, :len(part)]
    return out.reshape(B, C, H, W), res


def kernel(background, foreground, mask):
    out, _ = _run(background, foreground, mask)
    return out
